# revision 44
# baseline (speedup 1.0000x reference)
"""Trainium2 Bass kernel for nn_LCNNConvolution (GNN message passing).

Math:  out[n] = sum_p softplus( gather(X, NS[n,p,:]).flat @ W.T + b ) - 12*ln2
Key transform: W is block-structured over the 8 neighbor slots, so
    x1[n,p,:] = sum_k Y_k[NS[n,p,k]]        with  Y_k = X @ W_k.T  (+ b baked
into slot 7). We precompute Y on-chip (PE matmul, f16), write it to DRAM as
[site, 8*64] f16 rows (1KB), then the hot loop is an indirect-DMA gather of
128B (64 x f16) row slices + DVE reduction over the 8 slots x 2 banks + ACT
softplus + DVE reduction over 12 perms.

v2 over baseline:
  - Y table in f16 -> 128B gather elements (the 256B elem_size constraint is
    transpose-only in the dma_gather ucode; we emit InstDMAGatherAnt directly)
    => gather HBM traffic halved.
  - phase-1 matmul in f16 (4x PE rate vs f32), Y write volume halved.
  - idx shipped as RAW uint16 site ids [chunk, 16, 512] (16x less wire than
    the replicated 2-bank int16 form); both bank index sets are derived
    on-device with one DVE umin / umax-sub pair and replicated to the 128
    SBUF partitions by DMA. X shipped f16, output returned f16.
  - phase-1 bias folded into the matmul via a 65th ones-row in lhsT, so each
    128-site block is a single PE instruction; PSUM drained alternately by
    ACT and DVE.

Sharding: data-parallel over sites; each of the 8 cores handles 6250 sites and
computes its own full Y copy (replicated X / W).
"""

import numpy as np

import jax

# Persistent XLA compilation cache: repeat kernel() calls (and fresh processes
# on the same machine) skip the retrace->walrus->NEFF pipeline (~0.7s/call).
try:
    jax.config.update("jax_compilation_cache_dir", "/tmp/jax_comp_cache")
    jax.config.update("jax_persistent_cache_min_entry_size_bytes", 0)
    jax.config.update("jax_persistent_cache_min_compile_time_secs", 0)
except Exception:
    pass

import concourse.bass as bass
import concourse.bacc as bacc
import concourse.mybir as mybir
import concourse.tile as tile
import concourse.ap_utils as ap_utils
from concourse.bass_utils import run_bass_kernel_spmd

# ---------------------------------------------------------------- constants
N_SITES = 50000
NODE_F = 64
N_PERM = 12
N_NEIGH = 8
OUT_F = 64
LN2 = float(np.log(2.0))

N_CORES = 8
SITES_PER_CORE = N_SITES // N_CORES            # 6250
SITES_PER_PART = 50                            # ceil(6250/128) padded to 50
PAD_SITES = 128 * SITES_PER_PART               # 6400
COLS = SITES_PER_PART * N_PERM                 # 600 rows (n,p) per partition
GCOLS = 8                                      # cols per dma_gather call
N_CHUNKS = COLS // GCOLS                       # 75 gather chunks
NIDX = 128 * GCOLS                             # 1024 gathers/call (HW limit)
RCOLS = 24                                     # cols per reduce group (2 sites)
BANK = 32767                                   # bank A covers sites [0, 32767)
DUMMY_B = 50001 - BANK                         # zero row for bank B

XT_PAD = 50176                                 # 392*128, padded site count
YROWS = N_SITES                                # Y table rows

F32 = mybir.dt.float32
F16 = mybir.dt.float16
I32 = mybir.dt.int32
I16 = mybir.dt.int16
U16 = mybir.dt.uint16



def dma_gather_f16(nc, out_ap, in_ap, idxs_ap, num_idxs, elem_size, elem_step,
                   queue_num=0):
    """Non-transpose dma_gather with a 128-byte element (64 x f16).

    Mirrors BassGpSimd.dma_gather's lowering, minus the `elem_size_bytes %
    256 == 0` assert, which the ucode (q7_kernels/extended_inst/dma_gather.cpp
    + decode/dma_gather.hpp) only imposes on the transpose path. The row
    stride (elem_step) must still be a multiple of 256 bytes because of the
    stride_bytes_256 descriptor encoding.
    """
    eng = nc.gpsimd
    assert idxs_ap.dtype == mybir.dt.int16
    assert in_ap.dtype == out_ap.dtype
    dt_size = mybir.dt.size(in_ap.dtype)
    elem_size_bytes = elem_size * dt_size
    assert elem_size_bytes > 0 and elem_size_bytes % 64 == 0
    assert ap_utils.ap_is_contiguous(in_ap.ap[1:])
    assert ap_utils.ap_is_contiguous(out_ap.ap[1:])
    assert ap_utils.ap_is_contiguous(idxs_ap.ap[1:])
    assert in_ap.ap[-1][1] == out_ap.ap[-1][1] == elem_size
    assert out_ap.ap[0][1] * out_ap.ap[1][1] == ((num_idxs + 127) // 128) * 128
    assert in_ap.ap[0][0] == elem_step
    stride_bytes = elem_step * dt_size
    assert stride_bytes % 256 == 0
    stride_bytes_256 = stride_bytes // 256
    assert 0 < stride_bytes_256 < 256

    _in_ap = eng.lower_ap_dma(in_ap, for_custom_bir_dma=True)
    _idxs_ap = eng.lower_ap(idxs_ap)
    _out_ap = eng.lower_ap(out_ap)
    return eng.add_instruction(
        mybir.InstDMAGatherAnt(
            name=nc.get_next_instruction_name(),
            ins=[
                *_in_ap,
                _idxs_ap,
                eng.lower_val_access(eng.to_reg(num_idxs)),
            ],
            outs=[_out_ap],
            transpose=False,
            num_idxs=num_idxs,
            elem_size=elem_size,
            stride_bytes_256=stride_bytes_256,
            gen_mode=0,
            single_packet=True,
            queue_num=queue_num,
            sbuf_tokens_per_rank=0,
            sbuf_free_dim_per_rank=0,
            sbuf_free_dim_pad_per_rank=0,
            sbuf_byte_offset=0,
        )
    )


# ---------------------------------------------------------------- device IR
def build_nc(skip_phase1=False, skip_gather=False, skip_reduce=False,
             p1_no_copy=False, p1_no_write=False):
    nc = bacc.Bacc("TRN2", target_bir_lowering=False, debug=False)

    # xt rows 0..63 = X.T (f16, zero-padded cols), row 64 = ones; wt rows
    # 0..63 = W in [feature, slot*64+out] layout, row 64 = bias (slot 7 only)
    xt = nc.dram_tensor("xt", [65, XT_PAD], F16, kind="ExternalInput").ap()
    wt = nc.dram_tensor("wt", [65, 512], F16, kind="ExternalInput").ap()
    # per chunk: 8 slot index sets of RAW uint16 site ids, 16-partition-wrapped
    # (compact; replicated to 128 partitions on-device). Bank indices are
    # derived on-device: idxA = umin(s, 32767) (row 32767 = zero dummy),
    # idxB = umax(s, 32766) - 32766 (row 0 = zero dummy).
    idx = nc.dram_tensor(
        "idx", [N_CHUNKS, 16, N_NEIGH * (NIDX // 16)], U16, kind="ExternalInput"
    ).ap()
    out = nc.dram_tensor(
        "out", [128, SITES_PER_PART, OUT_F], F16, kind="ExternalOutput"
    ).ap()

    with tile.TileContext(nc) as tc:
        with (
            tc.tile_pool(name="persist", bufs=1) as persist,
            tc.tile_pool(name="dram", bufs=1, space="DRAM") as dram,
        ):
            half_sb = persist.tile([128, 1], F32)
            nc.vector.memset(half_sb[:], 0.5)

            # Y table split at the int16 bank boundary:
            #   ybigA row s       = sites 0..32766,        row 32767 = zero
            #   ybigB row s-32766 = sites 32767..49999,    row 0     = zero
            ybigA = dram.tile([BANK + 1, 512], F16)
            ybigB = dram.tile([DUMMY_B, 512], F16)
            zrow = persist.tile([1, 512], F16)
            nc.vector.memset(zrow[:], 0.0)
            nc.sync.dma_start(out=ybigA[BANK : BANK + 1, :], in_=zrow[:])
            nc.sync.dma_start(out=ybigB[0:1, :], in_=zrow[:])

            # ---------------- phase 1: Y = X @ Wall.T  (+bias in slot 7)
            with (
                tc.tile_pool(name="p1", bufs=1) as p1,
                tc.tile_pool(name="p1y", bufs=6) as p1y,
                tc.tile_pool(name="p1ps", bufs=8, space="PSUM") as p1ps,
            ):
                xt_sb = p1.tile([65, XT_PAD], F16)
                nc.sync.dma_start(out=xt_sb[:], in_=xt[:])
                wt_sb = p1.tile([65, 512], F16)
                nc.sync.dma_start(out=wt_sb[:], in_=wt[:])

                for j in range(XT_PAD // 128):
                    if skip_phase1:
                        break
                    s0 = j * 128  # first site of this block
                    if s0 >= N_SITES:
                        break
                    nrows = min(128, N_SITES - s0)
                    psum = p1ps.tile([128, 512], F32, space="PSUM", tag="ps")
                    nc.tensor.matmul(
                        out=psum[:],
                        lhsT=xt_sb[:, j * 128 : (j + 1) * 128],
                        rhs=wt_sb[:],
                        start=True,
                        stop=True,
                    )
                    if p1_no_copy:
                        continue
                    y_sb = p1y.tile([128, 512], F16, tag="y")
                    # alternate the PSUM drain between ACT and DVE so the
                    # copy chain is not serialized on one engine
                    if j % 2 == 0:
                        nc.scalar.copy(out=y_sb[:], in_=psum[:])
                    else:
                        nc.vector.tensor_copy(out=y_sb[:], in_=psum[:])
                    if p1_no_write:
                        continue
                    a_hi = min(s0 + nrows, BANK)
                    if a_hi > s0:
                        nc.sync.dma_start(
                            out=ybigA[s0:a_hi, :],
                            in_=y_sb[: a_hi - s0, :],
                        )
                    b_lo = max(s0, BANK)
                    if s0 + nrows > b_lo:
                        nc.sync.dma_start(
                            out=ybigB[b_lo - BANK + 1 : s0 + nrows - BANK + 1, :],
                            in_=y_sb[b_lo - s0 : nrows, :],
                        )

            # ---------------- phase 2: dma_gather (2 banks x 8 slots) + reduce
            # For each (n,p) position and slot k with site s = NS[n,p,k]:
            # bank A gathers ybigA[umin(s, 32767)] (s>=32767 hits the zero
            # row), bank B gathers ybigB[umax(s, 32766)-32766] (s<=32766 hits
            # the zero row); x1 = sum over all 16 (slot, bank) gathers.
            with (
                tc.tile_pool(name="p2g", bufs=2) as p2g,
                tc.tile_pool(name="p2i", bufs=2) as p2i,
                tc.tile_pool(name="p2o", bufs=2) as p2o,
            ):
                x1 = None
                KW = N_NEIGH * (NIDX // 16)  # 512 raw idx cols per partition

                def prep_idx(j):
                    raw_sb = p2i.tile([128, KW], U16, tag="raw")
                    for r in range(8):
                        nc.sync.dma_start(
                            out=raw_sb[16 * r : 16 * r + 16, :], in_=idx[j]
                        )
                    iA = p2i.tile([128, KW], I16, tag="idxA")
                    nc.vector.tensor_scalar(
                        out=iA[:], in0=raw_sb[:], scalar1=BANK, scalar2=None,
                        op0=mybir.AluOpType.min,
                    )
                    iB = p2i.tile([128, KW], I16, tag="idxB")
                    nc.vector.tensor_scalar(
                        out=iB[:], in0=raw_sb[:], scalar1=BANK - 1,
                        scalar2=BANK - 1, op0=mybir.AluOpType.max,
                        op1=mybir.AluOpType.subtract,
                    )
                    return iA, iB

                # software-pipelined one chunk ahead: the DVE idx-prep of
                # chunk j+1 is issued before chunk j's reduce, so the gather
                # stream never stalls on the (in-order) DVE behind a reduce
                nxt = prep_idx(0)
                for j in range(N_CHUNKS):
                    idxA, idxB = nxt
                    if j + 1 < N_CHUNKS:
                        nxt = prep_idx(j + 1)
                    g = p2g.tile([128, 16, GCOLS, OUT_F], F16, tag="g")
                    for kb in range(16):
                        if skip_gather:
                            break
                        bank, k = kb // 8, kb % 8
                        if bank == 0:
                            tab = ybigA[:, k * 64 : (k + 1) * 64]
                            idx_sb = idxA
                        else:
                            tab = ybigB[:, k * 64 : (k + 1) * 64]
                            idx_sb = idxB
                        dma_gather_f16(
                            nc,
                            out_ap=g[:, kb, :, :],
                            in_ap=tab,
                            idxs_ap=idx_sb[
                                :, k * (NIDX // 16) : (k + 1) * (NIDX // 16)
                            ],
                            num_idxs=NIDX,
                            elem_size=64,
                            elem_step=512,
                        )
                    if skip_reduce:
                        continue
                    # x1[p, c, f] = sum_kb g[p, kb, c, f] into its 8-col slice
                    if j % 3 == 0:
                        x1 = p2o.tile([128, RCOLS, OUT_F], F32, tag="x1")
                    sub = j % 3
                    nc.vector.tensor_reduce(
                        out=x1[:, sub * GCOLS : (sub + 1) * GCOLS, :],
                        in_=g[:].rearrange("p k c f -> p c f k"),
                        axis=mybir.AxisListType.X,
                        op=mybir.AluOpType.add,
                    )
                    if sub != 2:
                        continue
                    grp = j // 3  # 24-col group = 2 sites
                    # softplus(x) - ln2 == Ln(0.5*Exp(x) + 0.5)
                    x2 = p2o.tile([128, RCOLS, OUT_F], F32, tag="x2")
                    nc.scalar.activation(
                        out=x2[:],
                        in_=x1[:],
                        func=mybir.ActivationFunctionType.Exp,
                    )
                    nc.scalar.activation(
                        out=x2[:],
                        in_=x2[:],
                        func=mybir.ActivationFunctionType.Ln,
                        scale=0.5,
                        bias=half_sb[:],
                    )
                    # out[p, s, f] = sum_q x2[p, s*12+q, f]  (f16 out: the
                    # 12-term sum of O(1) positives loses ~1e-3 rel, fine)
                    acc = p2o.tile([128, RCOLS // N_PERM, OUT_F], F16, tag="acc")
                    with nc.allow_low_precision(reason="f16 out within tolerance"):
                        nc.vector.tensor_reduce(
                            out=acc[:],
                            in_=x2[:].rearrange("p (s q) f -> p s f q", q=N_PERM),
                            axis=mybir.AxisListType.X,
                            op=mybir.AluOpType.add,
                        )
                    nc.sync.dma_start(
                        out=out[:, grp * 2 : grp * 2 + 2, :],
                        in_=acc[:],
                    )

    nc.compile()
    return nc


# ---------------------------------------------------------------- host side
def _host_prep(X_sites, X_NSs, W, b):
    X_sites = np.asarray(X_sites, dtype=np.float32)
    X_NSs = np.asarray(X_NSs)
    W = np.asarray(W, dtype=np.float32)
    b = np.asarray(b, dtype=np.float32)

    xt = np.zeros((65, XT_PAD), dtype=np.float16)
    xt[:64, :N_SITES] = X_sites.T
    xt[64, :] = 1.0

    wt = np.zeros((65, 512), dtype=np.float16)
    wt[:64] = W.reshape(OUT_F, N_NEIGH, NODE_F).transpose(2, 1, 0).reshape(NODE_F, 512)
    wt[64, 448:512] = b

    # raw uint16 site ids; bank splitting happens on-device (umin/umax-sub)
    s_all = X_NSs.astype(np.uint16)

    in_maps = []
    pad = PAD_SITES - SITES_PER_CORE
    for c in range(N_CORES):
        lo, hi = c * SITES_PER_CORE, (c + 1) * SITES_PER_CORE
        # V[p, cols, k]: pad sites use site 0 (harmless, rows discarded)
        V = np.concatenate(
            [s_all[lo:hi], np.zeros((pad, N_PERM, N_NEIGH), np.uint16)]
        ).reshape(128, COLS, N_NEIGH)
        # per call (chunk, k): position i = gcol*128 + p over 8 gcols
        arr = V.reshape(128, N_CHUNKS, GCOLS, N_NEIGH).transpose(1, 3, 2, 0)
        arr = arr.reshape(N_CHUNKS, N_NEIGH, NIDX)
        # 16-partition wrap: tile[p_row, col] = arr[col*16 + p_row]
        t16 = arr.reshape(N_CHUNKS, N_NEIGH, NIDX // 16, 16).transpose(0, 1, 3, 2)
        compact = np.ascontiguousarray(
            t16.transpose(0, 2, 1, 3).reshape(N_CHUNKS, 16, N_NEIGH * (NIDX // 16))
        )
        in_maps.append({"xt": xt, "wt": wt, "idx": compact})
    return in_maps


_NC_CACHE = {}


def _get_nc():
    if "nc" not in _NC_CACHE:
        _NC_CACHE["nc"] = build_nc()
    return _NC_CACHE["nc"]


def _stitch(results):
    full = np.empty((N_SITES, OUT_F), dtype=np.float32)
    for c, r in enumerate(results):
        o = r["out"].reshape(PAD_SITES, OUT_F)[:SITES_PER_CORE]
        full[c * SITES_PER_CORE : (c + 1) * SITES_PER_CORE] = o.astype(np.float32)
    return full


def kernel(X_sites, X_NSs, W, b, _trace=False):
    nc = _get_nc()
    in_maps = _host_prep(X_sites, X_NSs, W, b)
    res = run_bass_kernel_spmd(
        nc, in_maps, core_ids=list(range(N_CORES)), trace=_trace
    )
    full = _stitch(res.results)
    if _trace:
        return full, res
    return full

